# revision 9
# baseline (speedup 1.0000x reference)
"""Trainium2 Bass kernel for nn_AttentionModule (retrieval_knn).

reference math:
    S = support.reshape(B, N*K, D)
    dist_sq[b,q,nk] = -sum_d (S[b,nk,d] - query[b,q,d])^2
    qgw = softmax_K(tanh(mean_q dist_sq))          # (B,N,K,1)
    agg = sum_k support * qgw                      # (B,N,D)

The mean over q commutes with the squared-distance expansion:
    mean_q dist_sq[b,nk] = 2*S.qbar - ||S||^2 - mean_q ||q||^2
with qbar = mean_q query[b].  So the kernel only needs one streaming
pass over query (the memory-bound part) plus a tiny support-side tail.

Sharding: pure data parallel over B=4 episodes (cores 4-7 duplicate).
"""

import numpy as np

import concourse.bass as bass
import concourse.tile as tile
from concourse import mybir
from concourse.bass_utils import run_bass_kernel_spmd

B, NN, KK, Q, D = 4, 10, 5, 512, 1536
NK = NN * KK  # 50
QT = Q // 128  # 4 query tiles of 128 rows
F32 = mybir.dt.float32
AF = mybir.ActivationFunctionType
ALU = mybir.AluOpType


def _split_multiwait(nc: bass.Bass) -> None:
    """This container's walrus only supports one sem-wait per instruction;
    Tile's final drain carries several. Hoist extra waits onto dedicated
    single-wait event-semaphore instructions right before the offender."""
    for f in nc.m.functions:
        for b in f.blocks:
            new_insts = []
            for ins in b.instructions:
                si = ins.sync_info
                if si is not None and len(si.on_wait) > 1:
                    waits = list(si.on_wait)
                    for i, w in enumerate(waits[:-1]):
                        ev = mybir.InstEventSemaphore(
                            name=f"{ins.name}-mw{i}",
                            engine=ins.engine,
                            sync_info=mybir.SyncInfo(on_wait=[w], on_update=[]),
                        )
                        new_insts.append(ev)
                    si.on_wait = waits[-1:]
                new_insts.append(ins)
            b.instructions[:] = new_insts


def _build_program(split_multiwait: bool = True) -> bass.Bass:
    nc = bass.Bass()

    q_in = nc.declare_dram_parameter("q", [Q, D], F32, isOutput=False)
    s_in = nc.declare_dram_parameter("s", [NK, D], F32, isOutput=False)
    g_in = nc.declare_dram_parameter("g", [NK, NN], F32, isOutput=False)
    gt_in = nc.declare_dram_parameter("gt", [NN, NK], F32, isOutput=False)
    agg_out = nc.declare_dram_parameter("agg", [NN, D], F32, isOutput=True)
    qgw_out = nc.declare_dram_parameter("qgw", [NK, 1], F32, isOutput=True)

    with tile.TileContext(nc) as tc:
        with (
            tc.tile_pool(name="consts", bufs=1) as consts,
            tc.tile_pool(name="qpool", bufs=QT) as qpool,
            tc.tile_pool(name="scr", bufs=2) as scr,
            tc.tile_pool(name="stats", bufs=1) as stats,
            tc.tile_pool(name="psum_qb", bufs=1, space="PSUM") as psum_qb_pool,
            tc.tile_pool(name="psum_small", bufs=2, space="PSUM") as psum_small,
            tc.tile_pool(name="outp", bufs=1) as outp,
        ):
            # --- issue all query-tile DMAs first (sync/HWDGE queue) ---
            qts = []
            for t in range(QT):
                qt = qpool.tile([128, D], F32, name="qt")
                nc.sync.dma_start(out=qt, in_=q_in[t * 128 : (t + 1) * 128, :])
                qts.append(qt)

            # --- support-side prologue on the gpsimd/SWDGE queue ---
            s_sb = consts.tile([NK, D], F32)
            nc.gpsimd.dma_start(out=s_sb, in_=s_in[:, :])
            g_sb = consts.tile([NK, NN], F32)
            nc.gpsimd.dma_start(out=g_sb, in_=g_in[:, :])
            gt_sb = consts.tile([NN, NK], F32)
            nc.gpsimd.dma_start(out=gt_sb, in_=gt_in[:, :])

            ones_b = consts.tile([128, NK], F32)  # lhsT: bcast+scale 2/Q
            nc.vector.memset(ones_b, 2.0 / Q)
            inv_qb = consts.tile([128, NK], F32)  # lhsT: bcast+scale 1/Q
            nc.vector.memset(inv_qb, 1.0 / Q)

            # ||S||^2 per support vector (ACT square+accumulate)
            sq_s = stats.tile([NK, 1], F32)
            junk0 = scr.tile([NK, D], F32, name="junk0")
            nc.scalar.activation(
                out=junk0, in_=s_sb, func=AF.Square, accum_out=sq_s
            )

            rowsq = stats.tile([128, QT], F32)  # per-query-row ||q||^2

            # --- streaming pass: ACT squares + DVE qsum accumulation ---
            qsum = stats.tile([128, D], F32)
            for t in range(QT):
                qt = qts[t]
                sq_scr = scr.tile([128, D], F32, name="sq_scr")
                nc.scalar.activation(
                    out=sq_scr,
                    in_=qt,
                    func=AF.Square,
                    accum_out=rowsq[:, t : t + 1],
                )
                if t == 1:
                    nc.vector.tensor_add(qsum, qts[0], qts[1])
                elif t > 1:
                    nc.vector.tensor_add(qsum, qsum, qt)

            # --- tail ---
            # qsum broadcast to NK partitions, scaled 2/Q (one matmul set)
            pqb = psum_qb_pool.tile([NK, D], F32, name="pqb", tag="pqb")
            for c in range(D // 512):
                nc.tensor.matmul(
                    pqb[:, c * 512 : (c + 1) * 512],
                    lhsT=ones_b,
                    rhs=qsum[:, c * 512 : (c + 1) * 512],
                    start=True,
                    stop=True,
                )

            # mean ||q||^2 broadcast to NK partitions (scaled 1/Q)
            pssq = psum_small.tile([NK, QT], F32, tag="small")
            nc.tensor.matmul(pssq, lhsT=inv_qb, rhs=rowsq, start=True, stop=True)
            ssq_r = stats.tile([NK, 1], F32)
            nc.vector.tensor_reduce(
                out=ssq_r, in_=pssq, axis=mybir.AxisListType.X, op=ALU.add
            )

            # dot[nk] = (2/Q) * sum_d S[nk,d] * qsum[d]
            junk = scr.tile([NK, D], F32, name="junk")
            nc.vector.tensor_mul(junk, s_sb, pqb)
            dotq = stats.tile([NK, 1], F32)
            nc.vector.tensor_reduce(
                out=dotq, in_=junk, axis=mybir.AxisListType.X, op=ALU.add
            )

            # md = 2*S.qbar - ||S||^2 - mean||q||^2
            md = stats.tile([NK, 1], F32)
            nc.vector.tensor_sub(md, dotq, sq_s)
            nc.vector.tensor_sub(md, md, ssq_r)

            th = stats.tile([NK, 1], F32)
            nc.scalar.activation(out=th, in_=md, func=AF.Tanh)
            ex = stats.tile([NK, 1], F32)
            nc.scalar.activation(out=ex, in_=th, func=AF.Exp)

            # group-of-K softmax denominator via block-one-hot matmuls
            pden = psum_small.tile([NN, 1], F32, tag="small")
            nc.tensor.matmul(pden, lhsT=g_sb, rhs=ex, start=True, stop=True)
            den_sb = stats.tile([NN, 1], F32)
            nc.vector.tensor_copy(den_sb, pden)
            pdenb = psum_small.tile([NK, 1], F32, tag="small")
            nc.tensor.matmul(pdenb, lhsT=gt_sb, rhs=den_sb, start=True, stop=True)
            rden = stats.tile([NK, 1], F32)
            nc.vector.reciprocal(rden, pdenb)

            w = stats.tile([NK, 1], F32)
            nc.vector.tensor_mul(w, ex, rden)
            nc.gpsimd.dma_start(out=qgw_out[:, :], in_=w)

            # weighted aggregation, pipelined per 512-chunk:
            # matmul -> PSUM->SBUF copy -> DMA out
            wg = stats.tile([NK, NN], F32)
            nc.vector.tensor_scalar(
                out=wg, in0=g_sb, scalar1=w, scalar2=None, op0=ALU.mult
            )
            agg_sb = outp.tile([NN, D], F32)
            for c in range(D // 512):
                sl = slice(c * 512, (c + 1) * 512)
                pagg = psum_small.tile([NN, 512], F32, tag="small", name="pagg")
                nc.tensor.matmul(
                    pagg, lhsT=wg, rhs=s_sb[:, sl], start=True, stop=True
                )
                nc.vector.tensor_copy(agg_sb[:, sl], pagg)
                nc.sync.dma_start(out=agg_out[:, sl], in_=agg_sb[:, sl])

    if split_multiwait:
        _split_multiwait(nc)
    return nc


_NC_CACHE: bass.Bass | None = None


def _get_nc() -> bass.Bass:
    global _NC_CACHE
    if _NC_CACHE is None:
        _NC_CACHE = _build_program()
    return _NC_CACHE


def _host_inputs(support: np.ndarray, query: np.ndarray):
    g = np.zeros((NK, NN), dtype=np.float32)
    for n in range(NN):
        g[n * KK : (n + 1) * KK, n] = 1.0
    gt = np.ascontiguousarray(g.T)
    in_maps = []
    for core in range(8):
        b = core % B
        in_maps.append(
            {
                "q": np.ascontiguousarray(query[b], dtype=np.float32),
                "s": np.ascontiguousarray(
                    support[b].reshape(NK, D), dtype=np.float32
                ),
                "g": g,
                "gt": gt,
            }
        )
    return in_maps


def kernel(support, query, N=NN, K=KK, total_Q=Q, _trace=False):
    support = np.asarray(support, dtype=np.float32)
    query = np.asarray(query, dtype=np.float32)
    assert support.shape == (B, NN, KK, D)
    assert query.shape == (B, Q, D)

    nc = _get_nc()
    in_maps = _host_inputs(support, query)
    res = run_bass_kernel_spmd(nc, in_maps, list(range(8)), trace=_trace)

    agg = np.stack([res.results[b]["agg"] for b in range(B)])
    qgw = np.stack([res.results[b]["qgw"] for b in range(B)])
    out_agg = agg.astype(np.float32)
    out_qgw = qgw.reshape(B, NN, KK, 1).astype(np.float32)
    if _trace:
        return (out_agg, out_qgw), res
    return out_agg, out_qgw


# revision 10
# speedup vs baseline: 1.1308x; 1.1308x over previous
"""Trainium2 Bass kernel for nn_AttentionModule (retrieval_knn).

reference math:
    S = support.reshape(B, N*K, D)
    dist_sq[b,q,nk] = -sum_d (S[b,nk,d] - query[b,q,d])^2
    qgw = softmax_K(tanh(mean_q dist_sq))          # (B,N,K,1)
    agg = sum_k support * qgw                      # (B,N,D)

The mean over q commutes with the squared-distance expansion:
    mean_q dist_sq[b,nk] = 2*S.qbar - ||S||^2 - mean_q ||q||^2
with qbar = mean_q query[b].  So the kernel only needs one streaming
pass over query (the memory-bound part) plus a tiny support-side tail.

Sharding: pure data parallel over B=4 episodes (cores 4-7 duplicate).
"""

import numpy as np

import concourse.bass as bass
import concourse.tile as tile
from concourse import mybir
from concourse.bass_utils import run_bass_kernel_spmd

B, NN, KK, Q, D = 4, 10, 5, 512, 1536
NK = NN * KK  # 50
QT = Q // 128  # 4 query tiles of 128 rows
F32 = mybir.dt.float32
AF = mybir.ActivationFunctionType
ALU = mybir.AluOpType


def _split_multiwait(nc: bass.Bass) -> None:
    """This container's walrus only supports one sem-wait per instruction;
    Tile's final drain carries several. Hoist extra waits onto dedicated
    single-wait event-semaphore instructions right before the offender."""
    for f in nc.m.functions:
        for b in f.blocks:
            new_insts = []
            for ins in b.instructions:
                si = ins.sync_info
                if si is not None and len(si.on_wait) > 1:
                    waits = list(si.on_wait)
                    for i, w in enumerate(waits[:-1]):
                        ev = mybir.InstEventSemaphore(
                            name=f"{ins.name}-mw{i}",
                            engine=ins.engine,
                            sync_info=mybir.SyncInfo(on_wait=[w], on_update=[]),
                        )
                        new_insts.append(ev)
                    si.on_wait = waits[-1:]
                new_insts.append(ins)
            b.instructions[:] = new_insts


def _build_program(split_multiwait: bool = True) -> bass.Bass:
    nc = bass.Bass()

    q_in = nc.declare_dram_parameter("q", [Q, D], F32, isOutput=False)
    s_in = nc.declare_dram_parameter("s", [NK, D], F32, isOutput=False)
    g_in = nc.declare_dram_parameter("g", [NK, NN], F32, isOutput=False)
    gt_in = nc.declare_dram_parameter("gt", [NN, NK], F32, isOutput=False)
    agg_out = nc.declare_dram_parameter("agg", [NN, D], F32, isOutput=True)
    qgw_out = nc.declare_dram_parameter("qgw", [NK, 1], F32, isOutput=True)

    with tile.TileContext(nc) as tc:
        with (
            tc.tile_pool(name="consts", bufs=1) as consts,
            tc.tile_pool(name="qpool", bufs=QT) as qpool,
            tc.tile_pool(name="scr", bufs=2) as scr,
            tc.tile_pool(name="stats", bufs=1) as stats,
            tc.tile_pool(name="psum_qb", bufs=1, space="PSUM") as psum_qb_pool,
            tc.tile_pool(name="psum_small", bufs=2, space="PSUM") as psum_small,
            tc.tile_pool(name="outp", bufs=1) as outp,
        ):
            # --- issue all query-tile DMAs first (sync/HWDGE queue) ---
            qts = []
            for t in range(QT):
                qt = qpool.tile([128, D], F32, name="qt")
                nc.sync.dma_start(out=qt, in_=q_in[t * 128 : (t + 1) * 128, :])
                qts.append(qt)

            # --- support-side prologue on the gpsimd/SWDGE queue ---
            s_sb = consts.tile([NK, D], F32)
            nc.gpsimd.dma_start(out=s_sb, in_=s_in[:, :])
            g_sb = consts.tile([NK, NN], F32)
            nc.gpsimd.dma_start(out=g_sb, in_=g_in[:, :])
            gt_sb = consts.tile([NN, NK], F32)
            nc.gpsimd.dma_start(out=gt_sb, in_=gt_in[:, :])

            ones_b = consts.tile([128, NK], F32)  # lhsT: bcast+scale 2/Q
            nc.vector.memset(ones_b, 2.0 / Q)
            inv_qb = consts.tile([128, NK], F32)  # lhsT: bcast+scale 1/Q
            nc.vector.memset(inv_qb, 1.0 / Q)

            # ||S||^2 per support vector (ACT square+accumulate)
            sq_s = stats.tile([NK, 1], F32)
            junk0 = scr.tile([NK, D], F32, name="junk0")
            nc.scalar.activation(
                out=junk0, in_=s_sb, func=AF.Square, accum_out=sq_s
            )

            rowsq = stats.tile([128, QT], F32)  # per-query-row ||q||^2

            # --- streaming pass: ACT squares + DVE qsum accumulation ---
            qsum = stats.tile([128, D], F32)
            for t in range(QT):
                qt = qts[t]
                sq_scr = scr.tile([128, D], F32, name="sq_scr")
                nc.scalar.activation(
                    out=sq_scr,
                    in_=qt,
                    func=AF.Square,
                    accum_out=rowsq[:, t : t + 1],
                )
                if t == 1:
                    nc.vector.tensor_add(qsum, qts[0], qts[1])
                elif t > 1:
                    nc.vector.tensor_add(qsum, qsum, qt)

            # --- tail ---
            # qsum broadcast to NK partitions, scaled 2/Q (one matmul set)
            pqb = psum_qb_pool.tile([NK, D], F32, name="pqb", tag="pqb")
            for c in range(D // 512):
                nc.tensor.matmul(
                    pqb[:, c * 512 : (c + 1) * 512],
                    lhsT=ones_b,
                    rhs=qsum[:, c * 512 : (c + 1) * 512],
                    start=True,
                    stop=True,
                )

            # mean ||q||^2 broadcast to NK partitions (scaled 1/Q)
            pssq = psum_small.tile([NK, QT], F32, tag="small")
            nc.tensor.matmul(pssq, lhsT=inv_qb, rhs=rowsq, start=True, stop=True)
            ssq_r = stats.tile([NK, 1], F32)
            nc.vector.tensor_reduce(
                out=ssq_r, in_=pssq, axis=mybir.AxisListType.X, op=ALU.add
            )

            # dot[nk] = (2/Q) * sum_d S[nk,d] * qsum[d]
            junk = scr.tile([NK, D], F32, name="junk")
            nc.vector.tensor_mul(junk, s_sb, pqb)
            dotq = stats.tile([NK, 1], F32)
            nc.vector.tensor_reduce(
                out=dotq, in_=junk, axis=mybir.AxisListType.X, op=ALU.add
            )

            # md = 2*S.qbar - ||S||^2 - mean||q||^2
            md = stats.tile([NK, 1], F32)
            nc.vector.tensor_sub(md, dotq, sq_s)
            nc.vector.tensor_sub(md, md, ssq_r)

            th = stats.tile([NK, 1], F32)
            nc.scalar.activation(out=th, in_=md, func=AF.Tanh)
            ex = stats.tile([NK, 1], F32)
            nc.scalar.activation(out=ex, in_=th, func=AF.Exp)

            # group-of-K softmax denominator via block-one-hot matmuls
            pden = psum_small.tile([NN, 1], F32, tag="small")
            nc.tensor.matmul(pden, lhsT=g_sb, rhs=ex, start=True, stop=True)
            den_sb = stats.tile([NN, 1], F32)
            nc.vector.tensor_copy(den_sb, pden)
            pdenb = psum_small.tile([NK, 1], F32, tag="small")
            nc.tensor.matmul(pdenb, lhsT=gt_sb, rhs=den_sb, start=True, stop=True)
            rden = stats.tile([NK, 1], F32)
            nc.vector.reciprocal(rden, pdenb)

            w = stats.tile([NK, 1], F32)
            nc.vector.tensor_mul(w, ex, rden)
            nc.gpsimd.dma_start(out=qgw_out[:, :], in_=w)

            # weighted aggregation, pipelined per 512-chunk:
            # matmul -> PSUM->SBUF copy -> DMA out
            wg = stats.tile([NK, NN], F32)
            nc.vector.tensor_scalar(
                out=wg, in0=g_sb, scalar1=w, scalar2=None, op0=ALU.mult
            )
            agg_sb = outp.tile([NN, D], F32)
            for c in range(D // 512):
                sl = slice(c * 512, (c + 1) * 512)
                pagg = psum_small.tile([NN, 512], F32, tag="small", name="pagg")
                nc.tensor.matmul(
                    pagg, lhsT=wg, rhs=s_sb[:, sl], start=True, stop=True
                )
                nc.vector.tensor_copy(agg_sb[:, sl], pagg)
                nc.sync.dma_start(out=agg_out[:, sl], in_=agg_sb[:, sl])

    if split_multiwait:
        _split_multiwait(nc)
    return nc


_NC_CACHE: bass.Bass | None = None


def _get_nc() -> bass.Bass:
    global _NC_CACHE
    if _NC_CACHE is None:
        _NC_CACHE = _build_program()
    return _NC_CACHE


def _host_inputs(support: np.ndarray, query: np.ndarray):
    g = np.zeros((NK, NN), dtype=np.float32)
    for n in range(NN):
        g[n * KK : (n + 1) * KK, n] = 1.0
    gt = np.ascontiguousarray(g.T)
    m = (g @ g.T).astype(np.float32)  # block matrix: 1 within each group
    in_maps = []
    for core in range(8):
        b = core % B
        in_maps.append(
            {
                "q": np.ascontiguousarray(query[b], dtype=np.float32),
                "s": np.ascontiguousarray(
                    support[b].reshape(NK, D), dtype=np.float32
                ),
                "g": g,
                "gt": gt,
                "m": m,
            }
        )
    return in_maps


def kernel(support, query, N=NN, K=KK, total_Q=Q, _trace=False):
    support = np.asarray(support, dtype=np.float32)
    query = np.asarray(query, dtype=np.float32)
    assert support.shape == (B, NN, KK, D)
    assert query.shape == (B, Q, D)

    nc = _get_nc()
    in_maps = _host_inputs(support, query)
    res = run_bass_kernel_spmd(nc, in_maps, list(range(8)), trace=_trace)

    agg = np.stack([res.results[b]["agg"] for b in range(B)])
    qgw = np.stack([res.results[b]["qgw"] for b in range(B)])
    out_agg = agg.astype(np.float32)
    out_qgw = qgw.reshape(B, NN, KK, 1).astype(np.float32)
    if _trace:
        return (out_agg, out_qgw), res
    return out_agg, out_qgw


# revision 11
# speedup vs baseline: 1.2222x; 1.0808x over previous
"""Trainium2 Bass kernel for nn_AttentionModule (retrieval_knn).

reference math:
    S = support.reshape(B, N*K, D)
    dist_sq[b,q,nk] = -sum_d (S[b,nk,d] - query[b,q,d])^2
    qgw = softmax_K(tanh(mean_q dist_sq))          # (B,N,K,1)
    agg = sum_k support * qgw                      # (B,N,D)

The mean over q commutes with the squared-distance expansion:
    mean_q dist_sq[b,nk] = 2*S.qbar - ||S||^2 - mean_q ||q||^2
with qbar = mean_q query[b], so the kernel only streams query once
(the memory-bound part) plus a tiny support-side tail:

    pqb  = (2/Q) * colsum(query) broadcast to all NK support rows (PE,
           accumulated in PSUM straight from four 128-row DMA blocks)
    dot  = rowsum(S * pqb)            (DVE multiply + ACT accumulate)
    msq  = (1/Q) * sum ||q_row||^2    (ACT square+accumulate + PE bcast)
    md   = dot - ||S||^2 - msq
    qgw  = exp(tanh(md)) / groupsum   (group sums via a block matrix
                                       matmul; no max-subtraction needed
                                       since tanh output is in [-1,1])
    agg  = (G*qgw)^T @ S              (block-one-hot G as matmul lhsT)

Sharding: pure data parallel over the B=4 episodes on cores [0,2,4,6]
(one NeuronCore per HBM domain).  A per-episode kernel is raw Bass (no
Tile): this container's walrus rejects Tile's multi-wait drain, and the
hand schedule keeps the query stream unfragmented on the HWDGE queue.
"""

import numpy as np

from concourse.bass_utils import run_bass_kernel_spmd

from contextlib import ExitStack

import concourse.bass as bass
from concourse import mybir

B, NN, KK, Q, D = 4, 10, 5, 512, 1536
NK = NN * KK
F32 = mybir.dt.float32
AF = mybir.ActivationFunctionType
ALU = mybir.AluOpType

WARM1 = 3  # PE warm-up dummies before the first real matmul
WARM2 = 2  # PE dummies covering the DVE dot window


def _build_program() -> bass.Bass:
    nc = bass.Bass()

    q_in = nc.declare_dram_parameter("q", [Q, D], F32, isOutput=False)
    s_in = nc.declare_dram_parameter("s", [NK, D], F32, isOutput=False)
    g_in = nc.declare_dram_parameter("g", [NK, NN], F32, isOutput=False)
    m_in = nc.declare_dram_parameter("m", [NK, NK], F32, isOutput=False)
    agg_out = nc.declare_dram_parameter("agg", [NN, D], F32, isOutput=True)
    qgw_out = nc.declare_dram_parameter("qgw", [NK, 1], F32, isOutput=True)

    qsrc = [
        q_in[i * 128 : (i + 1) * 128, :].rearrange("(b p) d -> p b d", p=128)
        for i in range(4)
    ]

    with ExitStack() as ctx:
        E = ctx.enter_context
        qt = [E(nc.sbuf_tensor(f"qt{i}", [128, 1, D], F32)) for i in range(4)]
        sq_scr = [E(nc.sbuf_tensor(f"sqscr{i}", [128, 1, D], F32)) for i in range(4)]
        s_sb = E(nc.sbuf_tensor([NK, D], F32))
        junkS = E(nc.sbuf_tensor([NK, D], F32))
        g_sb = E(nc.sbuf_tensor([NK, NN], F32))
        m_sb = E(nc.sbuf_tensor([NK, NK], F32))
        ones_b = E(nc.sbuf_tensor([128, NK], F32))
        inv_qb = E(nc.sbuf_tensor([128, NK], F32))
        dummy_rhs = E(nc.sbuf_tensor([128, 512], F32))
        zeros = E(nc.sbuf_tensor([128, 1], F32))
        tblw = E(nc.sbuf_tensor([128, 1], F32))
        rowsq2 = E(nc.sbuf_tensor([128, 4], F32))
        junkD = E(nc.sbuf_tensor([NK, 3, 512], F32))
        junkA = E(nc.sbuf_tensor([NK, 3, 512], F32))
        dotacc = E(nc.sbuf_tensor([NK, 3], F32))
        dot3 = E(nc.sbuf_tensor([NK, 3], F32))
        sq_s = E(nc.sbuf_tensor([NK, 1], F32))
        ssq_r = E(nc.sbuf_tensor([NK, 1], F32))
        dotq = E(nc.sbuf_tensor([NK, 1], F32))
        md1 = E(nc.sbuf_tensor([NK, 1], F32))
        md = E(nc.sbuf_tensor([NK, 1], F32))
        th = E(nc.sbuf_tensor([NK, 1], F32))
        ex = E(nc.sbuf_tensor([NK, 1], F32))
        rden = E(nc.sbuf_tensor([NK, 1], F32))
        w_sb = E(nc.sbuf_tensor([NK, 1], F32))
        wg = E(nc.sbuf_tensor([NK, NN], F32))
        agg_sb = E(nc.sbuf_tensor([NN, D], F32))
        pqb = E(nc.psum_tensor([NK, D], F32))  # 3 banks
        pagg = E(nc.psum_tensor([NN, D], F32))  # 3 banks
        warm_ps = E(nc.psum_tensor([NK, 512], F32))  # 1 bank
        small_ps = E(nc.psum_tensor([NK, 128], F32))  # 1 bank
        SQT = [E(nc.semaphore(f"SQT{i}")) for i in range(4)]
        SLS = E(nc.semaphore("SLS"))  # s load
        SLG = E(nc.semaphore("SLG"))  # g+m loads
        SOUT = E(nc.semaphore("SOUT"))
        SGW = E(nc.semaphore("SGW"))
        SA = E(nc.semaphore("SA"))
        SV = E(nc.semaphore("SV"))
        SP = E(nc.semaphore("SP"))
        block = E(nc.Block())
        pssq = small_ps[:, 0:4]  # (50,4)
        pdenb = small_ps[:, 16:17]  # (50,1)

        @block.sync
        def _(sync):
            sync.dma_start(out=qt[0][:], in_=qsrc[0]).then_inc(SQT[0], 16)
            sync.dma_start(out=qt[1][:], in_=qsrc[1]).then_inc(SQT[1], 16)
            sync.dma_start(out=s_sb[:], in_=s_in[:, :]).then_inc(SLS, 16)
            sync.dma_start(out=qt[2][:], in_=qsrc[2]).then_inc(SQT[2], 16)
            sync.dma_start(out=g_sb[:], in_=g_in[:, :]).then_inc(SLG, 16)
            sync.dma_start(out=m_sb[:], in_=m_in[:, :]).then_inc(SLG, 16)
            sync.dma_start(out=qt[3][:], in_=qsrc[3]).then_inc(SQT[3], 16)
            sync.wait_ge(SV, 10)  # all agg chunks copied
            sync.dma_start(out=agg_out[:, :], in_=agg_sb[:]).then_inc(SOUT, 16)
            sync.wait_ge(SOUT, 16)

        @block.gpsimd
        def _(gpsimd):
            gpsimd.wait_ge(SV, 6)  # w ready
            gpsimd.dma_start(out=qgw_out[:, :], in_=w_sb[:]).then_inc(SGW, 16)
            gpsimd.wait_ge(SGW, 16)

        @block.scalar
        def _(scalar):
            scalar.wait_ge(SV, 1)  # zeros ready
            # trigger the ACT table-load DMA before the query stream owns
            # the DMA engines
            scalar.activation(
                out=tblw[:], in_=zeros[:], func=AF.Square, bias=zeros[:]
            )
            scalar.drain()
            scalar.wait_ge(SQT[0], 16)
            scalar.activation(
                out=sq_scr[0][:], in_=qt[0][:], func=AF.Square,
                bias=zeros[:], accum_out=rowsq2[:, 0:1],
            )
            scalar.wait_ge(SQT[1], 16)
            scalar.activation(
                out=sq_scr[1][:], in_=qt[1][:], func=AF.Square,
                bias=zeros[:], accum_out=rowsq2[:, 1:2],
            )
            scalar.wait_ge(SLS, 16)  # s loaded
            scalar.activation(
                out=junkS[:], in_=s_sb[:], func=AF.Square,
                bias=zeros[:NK], accum_out=sq_s[:],
            ).then_inc(SA, 2)  # SA=2
            scalar.wait_ge(SQT[2], 16)
            scalar.activation(
                out=sq_scr[2][:], in_=qt[2][:], func=AF.Square,
                bias=zeros[:], accum_out=rowsq2[:, 2:3],
            ).then_inc(SA, 1)  # SA=3
            scalar.wait_ge(SQT[3], 16)
            scalar.activation(
                out=sq_scr[3][:], in_=qt[3][:], func=AF.Square,
                bias=zeros[:], accum_out=rowsq2[:, 3:4],
            ).then_inc(SA, 1)  # SA=4
            # dot chunk reductions: plain copy with free-dim accumulate
            for c in range(3):
                scalar.wait_ge(SV, 2 + c)  # mul chunk c done
                scalar.activation(
                    out=junkA[:, c, :], in_=junkD[:, c, :], func=AF.Copy,
                    accum_out=dotacc[:, c : c + 1],
                ).then_inc(SA, 1)  # SA=5,6,7
            scalar.wait_ge(SV, 5)  # md ready
            scalar.activation(
                out=th[:], in_=md[:], func=AF.Tanh, bias=zeros[:NK]
            )
            scalar.drain()
            scalar.activation(
                out=ex[:], in_=th[:], func=AF.Exp, bias=zeros[:NK]
            ).then_inc(SA, 1)  # SA=8

        @block.vector
        def _(vector):
            vector.memset(zeros[:], 0.0)
            vector.memset(dummy_rhs[:], 1.0)
            vector.memset(ones_b[:], 2.0 / Q)
            vector.memset(inv_qb[:], 1.0 / Q).then_inc(SV, 1)  # SV=1
            vector.wait_ge(SLS, 16)  # s loaded
            # dot chunk multiplies; ACT does the accumulate-reduce
            for c in range(3):
                sl = slice(c * 512, (c + 1) * 512)
                vector.wait_ge(SP, 1 + c)
                vector.tensor_mul(
                    junkD[:, c, :], s_sb[:, sl], pqb[:, sl]
                ).then_inc(SV, 1)  # SV=2,3,4
            vector.wait_ge(SA, 7)  # dotacc complete
            vector.tensor_reduce(
                out=dotq[:], in_=dotacc[:], axis=mybir.AxisListType.X,
                op=ALU.add,
            )
            vector.wait_ge(SP, 4)  # pssq
            vector.tensor_reduce(
                out=ssq_r[:], in_=pssq, axis=mybir.AxisListType.X, op=ALU.add
            )
            vector.drain()
            vector.tensor_sub(md1[:], dotq[:], sq_s[:])
            vector.drain()
            vector.tensor_sub(md[:], md1[:], ssq_r[:]).then_inc(SV, 1)  # SV=5
            vector.wait_ge(SP, 5)  # pdenb (group sums)
            vector.reciprocal(rden[:], pdenb)
            vector.drain()
            vector.tensor_mul(w_sb[:], ex[:], rden[:]).then_inc(SV, 1)  # SV=6
            vector.drain()
            vector.wait_ge(SLG, 32)  # g loaded
            vector.tensor_scalar(
                out=wg[:], in0=g_sb[:], scalar1=w_sb[:], scalar2=None,
                op0=ALU.mult,
            ).then_inc(SV, 1)  # SV=7 (wg ready)
            for c in range(3):
                sl = slice(c * 512, (c + 1) * 512)
                vector.wait_ge(SP, 6 + c)
                vector.tensor_copy(agg_sb[:, sl], pagg[:, sl]).then_inc(
                    SV, 1
                )  # SV=8,9,10

        @block.tensor
        def _(tensor):
            tensor.wait_ge(SV, 1)  # dummy_rhs/ones ready
            for i in range(WARM1):
                tensor.matmul(
                    warm_ps[:], lhsT=ones_b[:], rhs=dummy_rhs[:],
                    start=(i == 0), stop=(i == WARM1 - 1),
                )
            # pqb chunk-major accumulation over the four 128-row blocks;
            # chunk c's group stops at blk3 -> SP=1+c
            for blk in range(4):
                tensor.wait_ge(SQT[blk], 16)
                for c in range(3):
                    sl = slice(c * 512, (c + 1) * 512)
                    mm = tensor.matmul(
                        pqb[:, sl], lhsT=ones_b[:], rhs=qt[blk][:, 0, sl],
                        start=(blk == 0), stop=(blk == 3),
                    )
                    if blk == 3:
                        mm.then_inc(SP, 1)  # SP=1,2,3
            tensor.wait_ge(SA, 4)  # rowsq2 ready
            tensor.matmul(
                pssq, lhsT=inv_qb[:], rhs=rowsq2[:], start=True, stop=True
            ).then_inc(SP, 1)  # SP=4
            tensor.wait_ge(SP, 4)  # self-edge for warm group WAW
            for i in range(WARM2):
                tensor.matmul(
                    warm_ps[:], lhsT=ones_b[:], rhs=dummy_rhs[:],
                    start=(i == 0), stop=(i == WARM2 - 1),
                )
            tensor.wait_ge(SLG, 32)  # g+m loaded
            tensor.wait_ge(SA, 8)  # ex ready
            tensor.matmul(
                pdenb, lhsT=m_sb[:], rhs=ex[:], start=True, stop=True
            ).then_inc(SP, 1)  # SP=5 (per-group sums broadcast)
            tensor.wait_ge(SV, 7)  # wg ready
            for c in range(3):
                sl = slice(c * 512, (c + 1) * 512)
                tensor.matmul(
                    pagg[:, sl], lhsT=wg[:], rhs=s_sb[:, sl],
                    start=True, stop=True,
                ).then_inc(SP, 1)  # SP=6,7,8

    return nc


_NC_CACHE = None


def _get_nc():
    global _NC_CACHE
    if _NC_CACHE is None:
        _NC_CACHE = _build_program()
    return _NC_CACHE


CORE_IDS = [0, 2, 4, 6]  # one core per HBM domain


def _host_inputs(support: np.ndarray, query: np.ndarray):
    g = np.zeros((NK, NN), dtype=np.float32)
    for n in range(NN):
        g[n * KK : (n + 1) * KK, n] = 1.0
    m = (g @ g.T).astype(np.float32)  # 1 within each group of K
    in_maps = []
    for b in range(B):
        in_maps.append(
            {
                "q": np.ascontiguousarray(query[b], dtype=np.float32),
                "s": np.ascontiguousarray(
                    support[b].reshape(NK, D), dtype=np.float32
                ),
                "g": g,
                "m": m,
            }
        )
    return in_maps


def kernel(support, query, N=NN, K=KK, total_Q=Q, _trace=False):
    support = np.asarray(support, dtype=np.float32)
    query = np.asarray(query, dtype=np.float32)
    assert support.shape == (B, NN, KK, D)
    assert query.shape == (B, Q, D)

    nc = _get_nc()
    in_maps = _host_inputs(support, query)
    res = run_bass_kernel_spmd(nc, in_maps, CORE_IDS, trace=_trace)

    agg = np.stack([res.results[b]["agg"] for b in range(B)]).astype(np.float32)
    qgw = (
        np.stack([res.results[b]["qgw"] for b in range(B)])
        .reshape(B, NN, KK, 1)
        .astype(np.float32)
    )
    if _trace:
        return (agg, qgw), res
    return agg, qgw


# revision 12
# speedup vs baseline: 1.2564x; 1.0280x over previous
"""Trainium2 Bass kernel for nn_AttentionModule (retrieval_knn).

reference math:
    S = support.reshape(B, N*K, D)
    dist_sq[b,q,nk] = -sum_d (S[b,nk,d] - query[b,q,d])^2
    qgw = softmax_K(tanh(mean_q dist_sq))          # (B,N,K,1)
    agg = sum_k support * qgw                      # (B,N,D)

The mean over q commutes with the squared-distance expansion:
    mean_q dist_sq[b,nk] = 2*S.qbar - ||S||^2 - mean_q ||q||^2
with qbar = mean_q query[b], so the kernel only streams query once
(the memory-bound part) plus a tiny support-side tail:

    pqb  = (2/Q) * colsum(query) broadcast to all NK support rows (PE,
           accumulated in PSUM straight from four 128-row DMA blocks)
    dot  = rowsum(S * pqb)            (DVE multiply + ACT accumulate)
    msq  = (1/Q) * sum ||q_row||^2    (ACT square+accumulate + PE bcast)
    md   = dot - ||S||^2 - msq
    qgw  = exp(tanh(md)) / groupsum   (group sums via a block matrix
                                       matmul; no max-subtraction needed
                                       since tanh output is in [-1,1])
    agg  = (G*qgw)^T @ S              (block-one-hot G as matmul lhsT)

Sharding: pure data parallel over the B=4 episodes on cores [0,2,4,6]
(one NeuronCore per HBM domain).  A per-episode kernel is raw Bass (no
Tile): this container's walrus rejects Tile's multi-wait drain, and the
hand schedule keeps the query stream unfragmented on the HWDGE queue.
"""

import numpy as np

from concourse.bass_utils import run_bass_kernel_spmd

from contextlib import ExitStack

import concourse.bass as bass
from concourse import mybir

B, NN, KK, Q, D = 4, 10, 5, 512, 1536
NK = NN * KK
F32 = mybir.dt.float32
AF = mybir.ActivationFunctionType
ALU = mybir.AluOpType

WARM1 = 3  # PE warm-up dummies before the first real matmul
WARM2 = 10  # short (N=128) PE dummies covering the DVE dot window


def _build_program() -> bass.Bass:
    nc = bass.Bass()

    q_in = nc.declare_dram_parameter("q", [Q, D], F32, isOutput=False)
    s_in = nc.declare_dram_parameter("s", [NK, D], F32, isOutput=False)
    g_in = nc.declare_dram_parameter("g", [NK, NN], F32, isOutput=False)
    m_in = nc.declare_dram_parameter("m", [NK, NK], F32, isOutput=False)
    agg_out = nc.declare_dram_parameter("agg", [NN, D], F32, isOutput=True)
    qgw_out = nc.declare_dram_parameter("qgw", [NK, 1], F32, isOutput=True)

    qsrc = [
        q_in[i * 128 : (i + 1) * 128, :].rearrange("(b p) d -> p b d", p=128)
        for i in range(4)
    ]

    with ExitStack() as ctx:
        E = ctx.enter_context
        qt = [E(nc.sbuf_tensor(f"qt{i}", [128, 1, D], F32)) for i in range(4)]
        sq_scr = [E(nc.sbuf_tensor(f"sqscr{i}", [128, 1, D], F32)) for i in range(4)]
        s_sb = E(nc.sbuf_tensor([NK, D], F32))
        junkS = E(nc.sbuf_tensor([NK, D], F32))
        g_sb = E(nc.sbuf_tensor([NK, NN], F32))
        m_sb = E(nc.sbuf_tensor([NK, NK], F32))
        ones_b = E(nc.sbuf_tensor([128, NK], F32))
        inv_qb = E(nc.sbuf_tensor([128, NK], F32))
        dummy_rhs = E(nc.sbuf_tensor([128, 512], F32))
        zeros = E(nc.sbuf_tensor([128, 1], F32))
        tblw = E(nc.sbuf_tensor([128, 1], F32))
        rowsq2 = E(nc.sbuf_tensor([128, 4], F32))
        junkD = E(nc.sbuf_tensor([NK, 3, 512], F32))
        junkA = E(nc.sbuf_tensor([NK, 3, 512], F32))
        dotacc = E(nc.sbuf_tensor([NK, 3], F32))
        dot3 = E(nc.sbuf_tensor([NK, 3], F32))
        sq_s = E(nc.sbuf_tensor([NK, 1], F32))
        ssq_r = E(nc.sbuf_tensor([NK, 1], F32))
        dotq = E(nc.sbuf_tensor([NK, 1], F32))
        md1 = E(nc.sbuf_tensor([NK, 1], F32))
        md = E(nc.sbuf_tensor([NK, 1], F32))
        th = E(nc.sbuf_tensor([NK, 1], F32))
        ex = E(nc.sbuf_tensor([NK, 1], F32))
        rden = E(nc.sbuf_tensor([NK, 1], F32))
        w_sb = E(nc.sbuf_tensor([NK, 1], F32))
        wg = E(nc.sbuf_tensor([NK, NN], F32))
        agg_sb = E(nc.sbuf_tensor([NN, D], F32))
        pqb = E(nc.psum_tensor([NK, D], F32))  # 3 banks
        pagg = E(nc.psum_tensor([NN, D], F32))  # 3 banks
        warm_ps = E(nc.psum_tensor([NK, 512], F32))  # 1 bank
        small_ps = E(nc.psum_tensor([NK, 128], F32))  # 1 bank
        SQT = [E(nc.semaphore(f"SQT{i}")) for i in range(4)]
        SLS = E(nc.semaphore("SLS"))  # s load
        SLG = E(nc.semaphore("SLG"))  # g+m loads
        SOUT = E(nc.semaphore("SOUT"))
        SGW = E(nc.semaphore("SGW"))
        SA = E(nc.semaphore("SA"))
        SV = E(nc.semaphore("SV"))
        SP = E(nc.semaphore("SP"))
        block = E(nc.Block())
        pssq = small_ps[:, 0:4]  # (50,4)
        pdenb = small_ps[:, 16:17]  # (50,1)

        @block.sync
        def _(sync):
            sync.dma_start(out=qt[0][:], in_=qsrc[0]).then_inc(SQT[0], 16)
            sync.dma_start(out=qt[1][:], in_=qsrc[1]).then_inc(SQT[1], 16)
            sync.dma_start(out=s_sb[:], in_=s_in[:, :]).then_inc(SLS, 16)
            sync.dma_start(out=qt[2][:], in_=qsrc[2]).then_inc(SQT[2], 16)
            sync.dma_start(out=g_sb[:], in_=g_in[:, :]).then_inc(SLG, 16)
            sync.dma_start(out=m_sb[:], in_=m_in[:, :]).then_inc(SLG, 16)
            sync.dma_start(out=qt[3][:], in_=qsrc[3]).then_inc(SQT[3], 16)
            sync.wait_ge(SV, 9)  # agg chunks 0,1 copied
            sync.dma_start(out=agg_out[:, 0:1024], in_=agg_sb[:, 0:1024]).then_inc(
                SOUT, 16
            )
            sync.wait_ge(SV, 10)  # agg chunk 2 copied
            sync.dma_start(
                out=agg_out[:, 1024:1536], in_=agg_sb[:, 1024:1536]
            ).then_inc(SOUT, 16)
            sync.wait_ge(SOUT, 32)

        @block.gpsimd
        def _(gpsimd):
            gpsimd.wait_ge(SV, 6)  # w ready
            gpsimd.dma_start(out=qgw_out[:, :], in_=w_sb[:]).then_inc(SGW, 16)
            gpsimd.wait_ge(SGW, 16)

        @block.scalar
        def _(scalar):
            scalar.wait_ge(SV, 1)  # zeros ready
            # trigger the ACT table-load DMA before the query stream owns
            # the DMA engines
            scalar.activation(
                out=tblw[:], in_=zeros[:], func=AF.Square, bias=zeros[:]
            )
            scalar.drain()
            scalar.wait_ge(SQT[0], 16)
            scalar.activation(
                out=sq_scr[0][:], in_=qt[0][:], func=AF.Square,
                bias=zeros[:], accum_out=rowsq2[:, 0:1],
            )
            scalar.wait_ge(SQT[1], 16)
            scalar.activation(
                out=sq_scr[1][:], in_=qt[1][:], func=AF.Square,
                bias=zeros[:], accum_out=rowsq2[:, 1:2],
            )
            scalar.wait_ge(SLS, 16)  # s loaded
            scalar.activation(
                out=junkS[:], in_=s_sb[:], func=AF.Square,
                bias=zeros[:NK], accum_out=sq_s[:],
            ).then_inc(SA, 2)  # SA=2
            scalar.wait_ge(SQT[2], 16)
            scalar.activation(
                out=sq_scr[2][:], in_=qt[2][:], func=AF.Square,
                bias=zeros[:], accum_out=rowsq2[:, 2:3],
            ).then_inc(SA, 1)  # SA=3
            scalar.wait_ge(SQT[3], 16)
            scalar.activation(
                out=sq_scr[3][:], in_=qt[3][:], func=AF.Square,
                bias=zeros[:], accum_out=rowsq2[:, 3:4],
            ).then_inc(SA, 1)  # SA=4
            # dot chunk reductions: plain copy with free-dim accumulate
            for c in range(3):
                scalar.wait_ge(SV, 2 + c)  # mul chunk c done
                scalar.activation(
                    out=junkA[:, c, :], in_=junkD[:, c, :], func=AF.Copy,
                    accum_out=dotacc[:, c : c + 1],
                ).then_inc(SA, 1)  # SA=5,6,7
            scalar.wait_ge(SV, 5)  # md ready
            scalar.activation(
                out=th[:], in_=md[:], func=AF.Tanh, bias=zeros[:NK]
            )
            scalar.drain()
            scalar.activation(
                out=ex[:], in_=th[:], func=AF.Exp, bias=zeros[:NK]
            ).then_inc(SA, 1)  # SA=8

        @block.vector
        def _(vector):
            vector.memset(zeros[:], 0.0)
            vector.memset(dummy_rhs[:], 1.0)
            vector.memset(ones_b[:], 2.0 / Q)
            vector.memset(inv_qb[:], 1.0 / Q).then_inc(SV, 1)  # SV=1
            vector.wait_ge(SLS, 16)  # s loaded
            # dot chunk multiplies; ACT does the accumulate-reduce
            for c in range(3):
                sl = slice(c * 512, (c + 1) * 512)
                vector.wait_ge(SP, 1 + c)
                vector.tensor_mul(
                    junkD[:, c, :], s_sb[:, sl], pqb[:, sl]
                ).then_inc(SV, 1)  # SV=2,3,4
            vector.wait_ge(SA, 7)  # dotacc complete
            vector.tensor_reduce(
                out=dotq[:], in_=dotacc[:], axis=mybir.AxisListType.X,
                op=ALU.add,
            )
            vector.wait_ge(SP, 4)  # pssq
            vector.tensor_reduce(
                out=ssq_r[:], in_=pssq, axis=mybir.AxisListType.X, op=ALU.add
            )
            vector.drain()
            vector.tensor_sub(md1[:], dotq[:], sq_s[:])
            vector.drain()
            vector.tensor_sub(md[:], md1[:], ssq_r[:]).then_inc(SV, 1)  # SV=5
            vector.wait_ge(SP, 5)  # pdenb (group sums)
            vector.reciprocal(rden[:], pdenb)
            vector.drain()
            vector.tensor_mul(w_sb[:], ex[:], rden[:]).then_inc(SV, 1)  # SV=6
            vector.drain()
            vector.wait_ge(SLG, 32)  # g loaded
            vector.tensor_scalar(
                out=wg[:], in0=g_sb[:], scalar1=w_sb[:], scalar2=None,
                op0=ALU.mult,
            ).then_inc(SV, 1)  # SV=7 (wg ready)
            for c in range(3):
                sl = slice(c * 512, (c + 1) * 512)
                vector.wait_ge(SP, 6 + c)
                vector.tensor_copy(agg_sb[:, sl], pagg[:, sl]).then_inc(
                    SV, 1
                )  # SV=8,9,10

        @block.tensor
        def _(tensor):
            tensor.wait_ge(SV, 1)  # dummy_rhs/ones ready
            for i in range(WARM1):
                tensor.matmul(
                    warm_ps[:], lhsT=ones_b[:], rhs=dummy_rhs[:],
                    start=(i == 0), stop=(i == WARM1 - 1),
                )
            # pqb chunk-major accumulation over the four 128-row blocks;
            # chunk c's group stops at blk3 -> SP=1+c
            for blk in range(4):
                tensor.wait_ge(SQT[blk], 16)
                for c in range(3):
                    sl = slice(c * 512, (c + 1) * 512)
                    mm = tensor.matmul(
                        pqb[:, sl], lhsT=ones_b[:], rhs=qt[blk][:, 0, sl],
                        start=(blk == 0), stop=(blk == 3),
                    )
                    if blk == 3:
                        mm.then_inc(SP, 1)  # SP=1,2,3
            tensor.wait_ge(SA, 4)  # rowsq2 ready
            tensor.matmul(
                pssq, lhsT=inv_qb[:], rhs=rowsq2[:], start=True, stop=True
            ).then_inc(SP, 1)  # SP=4
            tensor.wait_ge(SP, 4)  # self-edge for warm group WAW
            for i in range(WARM2):
                tensor.matmul(
                    warm_ps[:, 0:128], lhsT=ones_b[:], rhs=dummy_rhs[:, 0:128],
                    start=(i == 0), stop=(i == WARM2 - 1),
                )
            tensor.wait_ge(SLG, 32)  # g+m loaded
            tensor.wait_ge(SA, 8)  # ex ready
            tensor.matmul(
                pdenb, lhsT=m_sb[:], rhs=ex[:], start=True, stop=True
            ).then_inc(SP, 1)  # SP=5 (per-group sums broadcast)
            tensor.wait_ge(SV, 7)  # wg ready
            for c in range(3):
                sl = slice(c * 512, (c + 1) * 512)
                tensor.matmul(
                    pagg[:, sl], lhsT=wg[:], rhs=s_sb[:, sl],
                    start=True, stop=True,
                ).then_inc(SP, 1)  # SP=6,7,8

    return nc


_NC_CACHE = None


def _get_nc():
    global _NC_CACHE
    if _NC_CACHE is None:
        _NC_CACHE = _build_program()
    return _NC_CACHE


CORE_IDS = [0, 2, 4, 6]  # one core per HBM domain


def _host_inputs(support: np.ndarray, query: np.ndarray):
    g = np.zeros((NK, NN), dtype=np.float32)
    for n in range(NN):
        g[n * KK : (n + 1) * KK, n] = 1.0
    m = (g @ g.T).astype(np.float32)  # 1 within each group of K
    in_maps = []
    for b in range(B):
        in_maps.append(
            {
                "q": np.ascontiguousarray(query[b], dtype=np.float32),
                "s": np.ascontiguousarray(
                    support[b].reshape(NK, D), dtype=np.float32
                ),
                "g": g,
                "m": m,
            }
        )
    return in_maps


def kernel(support, query, N=NN, K=KK, total_Q=Q, _trace=False):
    support = np.asarray(support, dtype=np.float32)
    query = np.asarray(query, dtype=np.float32)
    assert support.shape == (B, NN, KK, D)
    assert query.shape == (B, Q, D)

    nc = _get_nc()
    in_maps = _host_inputs(support, query)
    res = run_bass_kernel_spmd(nc, in_maps, CORE_IDS, trace=_trace)

    agg = np.stack([res.results[b]["agg"] for b in range(B)]).astype(np.float32)
    qgw = (
        np.stack([res.results[b]["qgw"] for b in range(B)])
        .reshape(B, NN, KK, 1)
        .astype(np.float32)
    )
    if _trace:
        return (agg, qgw), res
    return agg, qgw
